# revision 1
# baseline (speedup 1.0000x reference)
"""Trainium2 Bass kernel for nn_MinifloatLinear.

Computes y = x @ quantize(W)^T + quantize(b) where quantize(W) is the
fp8 round-trip (e5m2 then e4m3fn) the module applies at construction
time, and quantize(b) is the e4m3fn round-trip for the bias.

Distribution: data-parallel over rows. x is [4, 2048, 4096] -> flattened
to [8192, 4096] and split into 8 shards of 1024 rows, one per NeuronCore.
Every core holds the full (quantized, bf16, pre-transposed) weight and
bias and produces its own 1024-row slab of the output.

Host-side prep (construction-time / layout-only work):
  - W -> e5m2 -> e4m3fn -> bf16 (exact: e4m3fn values are representable
    in bf16), then transposed to [in, out] so the device can DMA
    contraction-major tiles directly.
  - b -> e4m3fn -> f32, broadcast to [128, 4096].
  - x shards are transposed to [in, rows] (pure layout, still f32; the
    f32 -> bf16 rounding happens on-device).

Device kernel (per core): y_shard[r, o] = sum_i xT[i, r] * wT[i, o] + b[o]
as a K-contracted tiled matmul: lhsT = xT tiles (cast to bf16 on DVE),
rhs = wT tiles, fp32 PSUM accumulation, bias added during PSUM->SBUF
eviction, fp32 output.
"""

import os
import sys

import numpy as np
import ml_dtypes

# concourse resolves via the container PYTHONPATH (axon-boot image);
# fall back to the /opt checkout when running outside that environment.
if "/opt/trn_rl_repo" not in sys.path:  # pragma: no cover
    sys.path.append("/opt/trn_rl_repo")

B, S, D_IN, D_OUT = 4, 2048, 4096, 4096
N_CORES = 8
ROWS = B * S  # 8192
ROWS_PER_CORE = ROWS // N_CORES  # 1024
P = 128

_CACHE = {}


def _build_program():
    """Build + compile the per-core Bass/Tile program (identical on all cores)."""
    if "nc" in _CACHE:
        return _CACHE["nc"]

    from contextlib import ExitStack

    import concourse.bacc as bacc
    import concourse.tile as tile
    import concourse.mybir as mybir
    from concourse.kernels.tile_matmul import (
        composable_matmul_tile_kernel,
        cast_to_type,
        dma_from_dram_kxm,
        dma_from_dram_kxn,
        dma_to_dram_mxn,
    )

    nc = bacc.Bacc(
        "TRN2",
        target_bir_lowering=False,
        debug=False,
        num_devices=N_CORES,
        enable_asserts=False,
    )

    xT = nc.dram_tensor(
        "xT", [D_IN, ROWS_PER_CORE], mybir.dt.float32, kind="ExternalInput"
    )
    wT = nc.dram_tensor("wT", [D_IN, D_OUT], mybir.dt.bfloat16, kind="ExternalInput")
    bb = nc.dram_tensor("bb", [P, D_OUT], mybir.dt.float32, kind="ExternalInput")
    y = nc.dram_tensor(
        "y", [ROWS_PER_CORE, D_OUT], mybir.dt.float32, kind="ExternalOutput"
    )

    K_TILES = D_IN // 512  # 8

    with tile.TileContext(nc) as tc, ExitStack() as ctx:
        const = ctx.enter_context(tc.tile_pool(name="const", bufs=1))
        bias_sb = const.tile([P, D_OUT], mybir.dt.float32)
        nc.sync.dma_start(bias_sb[:], bb.ap())

        # f32 x staging (transient) separate from the cached bf16 tiles.
        stage_pool = ctx.enter_context(tc.tile_pool(name="xstage", bufs=3))
        kxm_pool = ctx.enter_context(tc.tile_pool(name="kxm", bufs=K_TILES + 1))
        kxn_pool = ctx.enter_context(tc.tile_pool(name="kxn", bufs=K_TILES + 1))

        kxm_producer, kxm_shape = dma_from_dram_kxm(stage_pool, xT.ap())
        kxm_producer = cast_to_type(kxm_producer, kxm_pool, mybir.dt.bfloat16)
        kxn_producer, kxn_shape = dma_from_dram_kxn(kxn_pool, wT.ap())
        mxn_consumer = dma_to_dram_mxn(y.ap())

        def bias_add_reducer(nc, psum, sbuf, md):
            start = md.n_tile_idx * md.n_tile + md.n_subtile_idx * md.n_subtile
            nc.vector.tensor_add(
                out=sbuf[:, :, : md.n_slice_size],
                in0=psum[:, : md.n_slice_size],
                in1=bias_sb[:, start : start + md.n_slice_size],
            )

        composable_matmul_tile_kernel(
            tc=tc,
            kxm_shape=kxm_shape,
            kxn_shape=kxn_shape,
            output_type=mybir.dt.float32,
            kxm_producer=kxm_producer,
            kxn_producer=kxn_producer,
            mxn_consumer=mxn_consumer,
            mxn_subtile_reducer=bias_add_reducer,
            MATMUL_FREE_DIM=512,
            MAX_TILE_SIZE=512,
            MAX_K_TILE_SIZE=512,
            cache_tiles=True,
            psum_n_bufs=2,
        )

    nc.compile()
    _CACHE["nc"] = nc
    return nc


def _prep_inputs(x, weight, bias):
    x2 = np.ascontiguousarray(np.asarray(x, dtype=np.float32).reshape(ROWS, D_IN))
    w = np.asarray(weight, dtype=np.float32)
    b = np.asarray(bias, dtype=np.float32)

    # Construction-time fp8 parameter quantization (matches the module).
    wq = w.astype(ml_dtypes.float8_e5m2).astype(ml_dtypes.float8_e4m3fn)
    wT_bf16 = np.ascontiguousarray(wq.astype(ml_dtypes.bfloat16).T)  # [in, out]
    bq = b.astype(ml_dtypes.float8_e4m3fn).astype(np.float32)
    bb = np.ascontiguousarray(np.broadcast_to(bq[None, :], (P, D_OUT)))

    in_maps = []
    for c in range(N_CORES):
        shard = x2[c * ROWS_PER_CORE : (c + 1) * ROWS_PER_CORE]
        in_maps.append(
            {
                "xT": np.ascontiguousarray(shard.T),  # [in, rows] f32
                "wT": wT_bf16,
                "bb": bb,
            }
        )
    return in_maps


def kernel(x, weight, bias):
    from concourse import bass_utils

    nc = _build_program()
    in_maps = _prep_inputs(x, weight, bias)
    res = bass_utils.run_bass_kernel_spmd(nc, in_maps, core_ids=list(range(N_CORES)))
    y = np.concatenate([res.results[c]["y"] for c in range(N_CORES)], axis=0)
    return np.ascontiguousarray(y.reshape(B, S, D_OUT).astype(np.float32, copy=False))
